# revision 8
# baseline (speedup 1.0000x reference)
"""Tree-GRU (binary heap, L=128) Trainium2 kernel.

Strategy:
  - Data-parallel over batch: B=128 -> 16 examples per NeuronCore, 8 cores.
  - The tree is a fixed binary heap (node n children 2n+1/2n+2), so nodes are
    processed level-by-level: leaves (64..127) need no recurrent matmuls at
    all; then node 63 (single real child 127), then levels 5..0.
  - Everything on-chip is feature-major [feat, node*16+b]; weights are
    pre-transposed on the host so no on-chip transposes are needed.
  - bf16 matmuls with fp32 PSUM accumulation; x-projections for internal
    nodes staged through DRAM in bf16; leaf hiddens fused straight from PSUM.
"""
import sys

sys.path.insert(0, "/opt/trn_rl_repo")

import numpy as np
import ml_dtypes

L = 128
B = 128
I = 1024
H = 1024
NCORES = 8
BL = B // NCORES          # 16 examples per core
COLS = L * BL             # 2048 (node*16 + b)
KC = I // 128             # 8 contraction chunks
GM = 5 * H // 128         # 40 gate M-tiles
CM = H // 128             # 8 cell M-tiles
TM = GM + CM              # 48
INT_COLS = 64 * BL        # 1024 internal-node columns (nodes 0..63)

_BUILT = None


def _build_nc():
    import concourse.bass as bass
    import concourse.mybir as mybir
    import concourse.tile as tile
    from concourse.vector_clock import ScopedClock, VectorClock

    BF = mybir.dt.bfloat16
    F32 = mybir.dt.float32
    SIG = mybir.ActivationFunctionType.Sigmoid
    TANH = mybir.ActivationFunctionType.Tanh

    def _legalize_single_wait(nc):
        # The walrus in this container encodes at most ONE sync wait per
        # TPB instruction; Tile's wait assigner can attach several. Hoist
        # the extras onto fresh same-engine NoOps placed directly before
        # the over-subscribed instruction.
        cur = nc.cur_bb.bb.instructions
        for blk in nc.m.functions[0].blocks:
            lst = blk.instructions
            i = 0
            while i < len(lst):
                ins = lst[i]
                si = getattr(ins, "sync_info", None)
                ow = list(si.on_wait) if si is not None and si.on_wait else []
                if len(ow) > 1:
                    si.on_wait = [ow[-1]]
                    for w in ow[:-1]:
                        nop = nc.engines[ins.engine].nop()
                        nopins = nop.ins
                        popped = cur.pop()
                        assert popped.name == nopins.name
                        nopins.sync_info = mybir.SyncInfo(on_wait=[w], on_update=[])
                        lst.insert(i, nopins)
                        i += 1
                i += 1

    class TC(tile.TileContext):
        def _drain_and_barrier(self, tick_clock, wait_clock):
            super()._drain_and_barrier(tick_clock, wait_clock)
            _legalize_single_wait(self.nc)

    nc = bass.Bass(trn_type="TRN2")

    xT = nc.declare_dram_parameter("xT", [I, COLS], BF, isOutput=False)
    wih = nc.declare_dram_parameter("wih", [I, 6 * H], BF, isOutput=False)
    wgl = nc.declare_dram_parameter("wgl", [H, 5 * H], BF, isOutput=False)
    wgr = nc.declare_dram_parameter("wgr", [H, 5 * H], BF, isOutput=False)
    wcl = nc.declare_dram_parameter("wcl", [H, H], BF, isOutput=False)
    wcr = nc.declare_dram_parameter("wcr", [H, H], BF, isOutput=False)
    bias = nc.declare_dram_parameter("bias", [128, TM], F32, isOutput=False)
    hT = nc.declare_dram_parameter("hT", [H, COLS], F32, isOutput=True)

    xgd = nc.dram_tensor("xgd", [6 * H, INT_COLS], BF)
    ident_d = nc.inline_tensor(np.eye(128, dtype=ml_dtypes.bfloat16), name="ident")

    with TC(nc) as tc:
        with (
            tc.tile_pool(name="res", bufs=1) as res,
            tc.tile_pool(name="xtp", bufs=1) as xtp,
            tc.tile_pool(name="wst", bufs=3) as wst,
            tc.tile_pool(name="xio", bufs=4) as xio,
            tc.tile_pool(name="gat", bufs=1) as gat,
            tc.tile_pool(name="prd", bufs=1) as prd,
            tc.tile_pool(name="wrk", bufs=2) as wrk,
            tc.tile_pool(name="ps", bufs=8, space="PSUM") as ps,
        ):
            # ---------------- residents ----------------
            wgl_sb = res.tile([128, KC, 5 * H], BF, tag="wgl")
            for k in range(KC):
                nc.sync.dma_start(out=wgl_sb[:, k, :], in_=wgl[k * 128:(k + 1) * 128, :])
            wcl_sb = res.tile([128, KC, H], BF, tag="wcl")
            wcr_sb = res.tile([128, KC, H], BF, tag="wcr")
            for k in range(KC):
                nc.sync.dma_start(out=wcl_sb[:, k, :], in_=wcl[k * 128:(k + 1) * 128, :])
                nc.sync.dma_start(out=wcr_sb[:, k, :], in_=wcr[k * 128:(k + 1) * 128, :])
            bias_sb = res.tile([128, TM], F32, tag="bias")
            nc.sync.dma_start(out=bias_sb, in_=bias[:, :])
            ident_sb = res.tile([128, 128], BF, tag="ident")
            nc.sync.dma_start(out=ident_sb, in_=ident_d[:, :])
            x63_sb = res.tile([128, TM, BL], BF, tag="x63")
            h127 = res.tile([128, KC, BL], BF, tag="h127")
            # per-level hidden buffers (bf16), slot j of hbuf[d] = node (2^d-1)+j
            hbuf = {}
            for d in range(7):
                hbuf[d] = res.tile([128, KC, max(2 ** d, 1), BL], BF,
                                   tag=f"h{d}", name=f"hbuf{d}")

            def mtile_dma(m):
                wt = wst.tile([128, KC, 128], BF, tag="w")
                nc.sync.dma_start(
                    out=wt,
                    in_=wih[:, m * 128:(m + 1) * 128].rearrange("(k p) m -> p k m", p=128),
                )
                return wt

            # ---------------- phase A: leaves (nodes 64..127) ----------------
            # h_leaf = sigmoid(z) * tanh(c); no recurrent terms.
            for q in range(2):
                col0 = 1024 + 512 * q
                xt_t = xtp.tile([128, KC, 512], BF, tag="xt")
                nc.sync.dma_start(
                    out=xt_t,
                    in_=xT[:, col0:col0 + 512].rearrange("(k p) c -> p k c", p=128),
                )
                for f in range(CM):
                    wz = mtile_dma(32 + f)
                    pz = ps.tile([128, 512], F32, tag="ps")
                    for k in range(KC):
                        nc.tensor.matmul(pz, wz[:, k, :], xt_t[:, k, :],
                                         start=(k == 0), stop=(k == KC - 1))
                    wc_ = mtile_dma(40 + f)
                    pc = ps.tile([128, 512], F32, tag="ps")
                    for k in range(KC):
                        nc.tensor.matmul(pc, wc_[:, k, :], xt_t[:, k, :],
                                         start=(k == 0), stop=(k == KC - 1))
                    zs = wrk.tile([128, 512], F32, tag="zs", bufs=1)
                    nc.scalar.activation(zs, pz, SIG, bias=bias_sb[:, 32 + f:33 + f])
                    cs = wrk.tile([128, 512], F32, tag="cs", bufs=1)
                    nc.scalar.activation(cs, pc, TANH, bias=bias_sb[:, 40 + f:41 + f])
                    h_t = wrk.tile([128, 512], F32, tag="h")
                    nc.vector.tensor_mul(h_t, zs, cs)
                    nc.sync.dma_start(out=hT[f * 128:(f + 1) * 128, col0:col0 + 512], in_=h_t)
                    if q == 0:
                        # nodes 64..95 -> hbuf[6] slots 1..32
                        nc.vector.tensor_copy(
                            hbuf[6][:, f, 1:33, :],
                            h_t.rearrange("p (n b) -> p n b", b=BL),
                        )
                    else:
                        # nodes 96..126 -> slots 33..63 ; node 127 -> h127
                        nc.vector.tensor_copy(
                            hbuf[6][:, f, 33:64, :],
                            h_t[:, :496].rearrange("p (n b) -> p n b", b=BL),
                        )
                        nc.vector.tensor_copy(h127[:, f, :], h_t[:, 496:512])

            # ---------------- phase A: internal x-projections (nodes 0..63) ----------------
            # Emit the node 32..63 half first so the n63 stage can start early.
            for col0 in (512, 0):
                xt_t = xtp.tile([128, KC, 512], BF, tag="xt")
                nc.sync.dma_start(
                    out=xt_t,
                    in_=xT[:, col0:col0 + 512].rearrange("(k p) c -> p k c", p=128),
                )
                for m in range(TM):
                    wt = mtile_dma(m)
                    pm = ps.tile([128, 512], F32, tag="ps")
                    for k in range(KC):
                        nc.tensor.matmul(pm, wt[:, k, :], xt_t[:, k, :],
                                         start=(k == 0), stop=(k == KC - 1))
                    xg_sb = xio.tile([128, 512], BF, tag="xio")
                    nc.vector.tensor_copy(xg_sb, pm)
                    nc.sync.dma_start(out=xgd[m * 128:(m + 1) * 128, col0:col0 + 512], in_=xg_sb)
                    if col0 == 512:
                        # node 63 cols are 1008..1023 -> offset 496 in this half
                        nc.vector.tensor_copy(x63_sb[:, m, :], pm[:, 496:512])

            # ---------------- node 63 (lh = h127, rh = 0) ----------------
            rl = gat.tile([128, KC, 256], BF, tag="rl")
            zl = gat.tile([128, KC, 256], BF, tag="zl")
            zz = gat.tile([128, KC, 256], BF, tag="zz")
            gmap63 = {0: rl, 2: zl, 4: zz}
            for m in range(GM):
                g = m // 8
                if g in (1, 3):  # rr, zr multiply rh = 0
                    continue
                pm = ps.tile([128, BL], F32, tag="ps")
                for k in range(KC):
                    nc.tensor.matmul(pm, wgl_sb[:, k, m * 128:(m + 1) * 128], h127[:, k, :],
                                     start=(k == 0), stop=False)
                nc.tensor.matmul(pm, ident_sb, x63_sb[:, m, :], start=False, stop=True)
                nc.scalar.activation(gmap63[g][:, m % 8, :BL], pm, SIG,
                                     bias=bias_sb[:, m:m + 1])
            pl = prd.tile([128, KC, 256], BF, tag="pl")
            for k in range(KC):
                nc.vector.tensor_mul(pl[:, k, :BL], rl[:, k, :BL], h127[:, k, :])
            for f in range(CM):
                pm = ps.tile([128, BL], F32, tag="ps")
                for k in range(KC):
                    nc.tensor.matmul(pm, wcl_sb[:, k, f * 128:(f + 1) * 128], pl[:, k, :BL],
                                     start=(k == 0))
                nc.tensor.matmul(pm, ident_sb, x63_sb[:, 40 + f, :], start=False, stop=True)
                cell = wrk.tile([128, 256], F32, tag="cell")
                nc.scalar.activation(cell[:, :BL], pm, TANH, bias=bias_sb[:, 40 + f:41 + f])
                h_t = wrk.tile([128, 512], F32, tag="h")
                tmp = wrk.tile([128, 512], F32, tag="tmp")
                nc.vector.tensor_mul(h_t[:, :BL], zl[:, f, :BL], h127[:, f, :])
                nc.vector.tensor_mul(tmp[:, :BL], zz[:, f, :BL], cell[:, :BL])
                nc.vector.tensor_add(h_t[:, :BL], h_t[:, :BL], tmp[:, :BL])
                nc.sync.dma_start(out=hT[f * 128:(f + 1) * 128, 63 * BL:64 * BL], in_=h_t[:, :BL])
                nc.vector.tensor_copy(hbuf[6][:, f, 0, :], h_t[:, :BL])

            # ---------------- levels 5..0 ----------------
            for d in range(5, -1, -1):
                node0 = 2 ** d - 1
                n_nodes = 2 ** d
                child = hbuf[d + 1].rearrange("p k (n two) b -> p k n two b", two=2)
                n_splits = max(1, (n_nodes * BL) // 256) if n_nodes * BL >= 256 else 1
                nodes_per_split = (n_nodes + n_splits - 1) // n_splits
                for s in range(n_splits):
                    ns0 = s * nodes_per_split
                    ns = min(nodes_per_split, n_nodes - ns0)
                    C = ns * BL
                    c0 = (node0 + ns0) * BL  # global hT/xgd column base
                    rl = gat.tile([128, KC, 256], BF, tag="rl")
                    rr = gat.tile([128, KC, 256], BF, tag="rr")
                    zl = gat.tile([128, KC, 256], BF, tag="zl")
                    zr = gat.tile([128, KC, 256], BF, tag="zr")
                    zz = gat.tile([128, KC, 256], BF, tag="zz")
                    gmap = {0: rl, 1: rr, 2: zl, 3: zr, 4: zz}

                    def LH(k):
                        return child[:, k, ns0:ns0 + ns, 0, :]

                    def RH(k):
                        return child[:, k, ns0:ns0 + ns, 1, :]

                    for m in range(GM):
                        wgrt = wst.tile([128, KC, 128], BF, tag="w")
                        nc.sync.dma_start(
                            out=wgrt,
                            in_=wgr[:, m * 128:(m + 1) * 128].rearrange("(k p) m -> p k m", p=128),
                        )
                        pm = ps.tile([128, C], F32, tag="ps")
                        for k in range(KC):
                            nc.tensor.matmul(pm, wgl_sb[:, k, m * 128:(m + 1) * 128], LH(k),
                                             start=(k == 0), stop=False)
                        for k in range(KC):
                            nc.tensor.matmul(pm, wgrt[:, k, :], RH(k), start=False, stop=False)
                        xgt = xio.tile([128, 512], BF, tag="xio")
                        nc.sync.dma_start(out=xgt[:, :C], in_=xgd[m * 128:(m + 1) * 128, c0:c0 + C])
                        nc.tensor.matmul(pm, ident_sb, xgt[:, :C], start=False, stop=True)
                        nc.scalar.activation(gmap[m // 8][:, m % 8, :C], pm, SIG,
                                             bias=bias_sb[:, m:m + 1])
                    pl = prd.tile([128, KC, 256], BF, tag="pl")
                    pr = prd.tile([128, KC, 256], BF, tag="pr")
                    for k in range(KC):
                        nc.vector.tensor_mul(
                            pl[:, k, :C].rearrange("p (n b) -> p n b", b=BL),
                            rl[:, k, :C].rearrange("p (n b) -> p n b", b=BL), LH(k))
                        nc.vector.tensor_mul(
                            pr[:, k, :C].rearrange("p (n b) -> p n b", b=BL),
                            rr[:, k, :C].rearrange("p (n b) -> p n b", b=BL), RH(k))
                    for f in range(CM):
                        pm = ps.tile([128, C], F32, tag="ps")
                        for k in range(KC):
                            nc.tensor.matmul(pm, wcl_sb[:, k, f * 128:(f + 1) * 128], pl[:, k, :C],
                                             start=(k == 0), stop=False)
                        for k in range(KC):
                            nc.tensor.matmul(pm, wcr_sb[:, k, f * 128:(f + 1) * 128], pr[:, k, :C], start=False, stop=False)
                        xct = xio.tile([128, 512], BF, tag="xio")
                        nc.sync.dma_start(out=xct[:, :C],
                                          in_=xgd[(40 + f) * 128:(41 + f) * 128, c0:c0 + C])
                        nc.tensor.matmul(pm, ident_sb, xct[:, :C], start=False, stop=True)
                        cell = wrk.tile([128, 256], F32, tag="cell")
                        nc.scalar.activation(cell[:, :C], pm, TANH, bias=bias_sb[:, 40 + f:41 + f])
                        h_t = wrk.tile([128, 512], F32, tag="h")
                        tmp = wrk.tile([128, 512], F32, tag="tmp")
                        nc.vector.tensor_mul(
                            h_t[:, :C].rearrange("p (n b) -> p n b", b=BL),
                            zl[:, f, :C].rearrange("p (n b) -> p n b", b=BL), LH(f))
                        nc.vector.tensor_mul(
                            tmp[:, :C].rearrange("p (n b) -> p n b", b=BL),
                            zr[:, f, :C].rearrange("p (n b) -> p n b", b=BL), RH(f))
                        nc.vector.tensor_add(h_t[:, :C], h_t[:, :C], tmp[:, :C])
                        nc.vector.tensor_mul(tmp[:, :C], zz[:, f, :C], cell[:, :C])
                        nc.vector.tensor_add(h_t[:, :C], h_t[:, :C], tmp[:, :C])
                        nc.sync.dma_start(out=hT[f * 128:(f + 1) * 128, c0:c0 + C],
                                          in_=h_t[:, :C])
                        nc.vector.tensor_copy(
                            hbuf[d][:, f, ns0:ns0 + ns, :],
                            h_t[:, :C].rearrange("p (n b) -> p n b", b=BL))
    return nc


def _build():
    global _BUILT
    if _BUILT is None:
        _BUILT = _build_nc()
    return _BUILT


def kernel(**inputs):
    inp = {k: np.asarray(v) for k, v in inputs.items()}
    bf = ml_dtypes.bfloat16
    x = inp["inputs"].astype(np.float32)  # [L, B, I]

    wih_np = np.ascontiguousarray(
        np.concatenate([inp["Wg_ih"], inp["Wc_ih"]], axis=0).T).astype(bf)  # [I, 6H]
    wgl_np = np.ascontiguousarray(inp["Wg_l"].T).astype(bf)   # [H, 5H]
    wgr_np = np.ascontiguousarray(inp["Wg_r"].T).astype(bf)
    wcl_np = np.ascontiguousarray(inp["Wc_l"].T).astype(bf)   # [H, H]
    wcr_np = np.ascontiguousarray(inp["Wc_r"].T).astype(bf)
    bias_np = np.ascontiguousarray(
        np.concatenate([inp["bg_ih"], inp["bc_ih"]]).astype(np.float32)
        .reshape(TM, 128).T)  # [128, 48]

    in_maps = []
    for c in range(NCORES):
        xs = x[:, c * BL:(c + 1) * BL, :]  # [L, BL, I]
        xTc = np.ascontiguousarray(xs.transpose(2, 0, 1).reshape(I, COLS)).astype(bf)
        in_maps.append(dict(xT=xTc, wih=wih_np, wgl=wgl_np, wgr=wgr_np,
                            wcl=wcl_np, wcr=wcr_np, bias=bias_np))

    nc = _build()
    from concourse.bass_utils import run_bass_kernel_spmd
    res = run_bass_kernel_spmd(nc, in_maps, list(range(NCORES)))

    parts = []
    for c in range(NCORES):
        hTc = np.asarray(res.results[c]["hT"], dtype=np.float32)  # [H, COLS]
        parts.append(hTc.reshape(H, L, BL).transpose(2, 1, 0))    # [BL, L, H]
    Hfin = np.ascontiguousarray(np.concatenate(parts, axis=0))    # [B, L, H]
    return Hfin, np.zeros((B, H), np.float32)


# revision 13
# speedup vs baseline: 1.2268x; 1.2268x over previous
"""Tree-GRU (binary heap, L=128) Trainium2 kernel.

Strategy:
  - Data-parallel over batch: B=128 -> 16 examples per NeuronCore, 8 cores.
  - The tree is a fixed binary heap (node n children 2n+1/2n+2), so nodes are
    processed level-by-level: leaves (64..127) need no recurrent matmuls at
    all; then node 63 (single real child 127), then levels 5..0.
  - Everything on-chip is feature-major [feat, node*16+b]; weights are
    pre-transposed on the host so no on-chip transposes are needed.
  - bf16 matmuls with fp32 PSUM accumulation; x-projections for internal
    nodes staged through DRAM in bf16; leaf hiddens fused straight from PSUM.
"""
import sys

sys.path.insert(0, "/opt/trn_rl_repo")

import numpy as np
import ml_dtypes

L = 128
B = 128
I = 1024
H = 1024
NCORES = 8
BL = B // NCORES          # 16 examples per core
COLS = L * BL             # 2048 (node*16 + b)
KC = I // 128             # 8 contraction chunks
GM = 5 * H // 128         # 40 gate M-tiles
CM = H // 128             # 8 cell M-tiles
TM = GM + CM              # 48
INT_COLS = 64 * BL        # 1024 internal-node columns (nodes 0..63)

_BUILT = None


def _build_nc():
    import concourse.bass as bass
    import concourse.mybir as mybir
    import concourse.tile as tile
    from concourse.vector_clock import ScopedClock, VectorClock

    BF = mybir.dt.bfloat16
    F32 = mybir.dt.float32
    SIG = mybir.ActivationFunctionType.Sigmoid
    TANH = mybir.ActivationFunctionType.Tanh

    def _legalize_single_wait(nc):
        # The walrus in this container encodes at most ONE sync wait per
        # TPB instruction; Tile's wait assigner can attach several. Hoist
        # the extras onto fresh same-engine NoOps placed directly before
        # the over-subscribed instruction.
        cur = nc.cur_bb.bb.instructions
        for blk in nc.m.functions[0].blocks:
            lst = blk.instructions
            i = 0
            while i < len(lst):
                ins = lst[i]
                si = getattr(ins, "sync_info", None)
                ow = list(si.on_wait) if si is not None and si.on_wait else []
                if len(ow) > 1:
                    si.on_wait = [ow[-1]]
                    for w in ow[:-1]:
                        nop = nc.engines[ins.engine].nop()
                        nopins = nop.ins
                        popped = cur.pop()
                        assert popped.name == nopins.name
                        nopins.sync_info = mybir.SyncInfo(on_wait=[w], on_update=[])
                        lst.insert(i, nopins)
                        i += 1
                i += 1

    class TC(tile.TileContext):
        def _drain_and_barrier(self, tick_clock, wait_clock):
            super()._drain_and_barrier(tick_clock, wait_clock)
            _legalize_single_wait(self.nc)

    nc = bass.Bass(trn_type="TRN2")

    xT = nc.declare_dram_parameter("xT", [4, 128, KC, 512], BF, isOutput=False)
    wih = nc.declare_dram_parameter("wih", [TM, 128, KC, 128], BF, isOutput=False)
    wgl = nc.declare_dram_parameter("wgl", [H, 5 * H], BF, isOutput=False)
    wgr = nc.declare_dram_parameter("wgr", [GM, 128, KC, 128], BF, isOutput=False)
    wcl = nc.declare_dram_parameter("wcl", [H, H], BF, isOutput=False)
    wcr = nc.declare_dram_parameter("wcr", [H, H], BF, isOutput=False)
    bias = nc.declare_dram_parameter("bias", [128, TM], F32, isOutput=False)
    hT = nc.declare_dram_parameter("hT", [H, COLS], F32, isOutput=True)

    xgd = nc.dram_tensor("xgd", [6 * H, INT_COLS], BF)

    with TC(nc) as tc:
        with (
            tc.tile_pool(name="res", bufs=1) as res,
            tc.tile_pool(name="xtp", bufs=2) as xtp,
            tc.tile_pool(name="wst", bufs=4) as wst,
            tc.tile_pool(name="xio", bufs=5) as xio,
            tc.tile_pool(name="gat", bufs=1) as gat,
            tc.tile_pool(name="wrk", bufs=2) as wrk,
            tc.tile_pool(name="ps", bufs=8, space="PSUM") as ps,
        ):
            # ---------------- residents ----------------
            wgl_sb = res.tile([128, KC, 5 * H], BF, tag="wgl")
            wcl_sb = res.tile([128, KC, H], BF, tag="wcl")
            wcr_sb = res.tile([128, KC, H], BF, tag="wcr")
            bias_sb = res.tile([128, TM], F32, tag="bias")
            nc.sync.dma_start(out=bias_sb, in_=bias[:, :])
            x63_sb = res.tile([128, TM, BL], BF, tag="x63")
            h127 = res.tile([128, KC, BL], BF, tag="h127")
            # per-level hidden buffers (bf16), slot j of hbuf[d] = node (2^d-1)+j
            hbuf = {}
            for d in range(7):
                hbuf[d] = res.tile([128, KC, max(2 ** d, 1), BL], BF,
                                   tag=f"h{d}", name=f"hbuf{d}")

            def mtile_dma(m):
                wt = wst.tile([128, KC, 128], BF, tag="w")
                nc.sync.dma_start(out=wt, in_=wih[m])
                return wt

            # ---------------- phase A: leaves (nodes 64..127) ----------------
            # h_leaf = sigmoid(z) * tanh(c); no recurrent terms. Single pass
            # over the z/c weight tiles, both 512-col quarters per tile.
            xt_leaf = []
            for q in range(2):
                xt_t = xtp.tile([128, KC, 512], BF, tag="xt", name=f"xtl{q}")
                nc.sync.dma_start(out=xt_t, in_=xT[2 + q])
                xt_leaf.append(xt_t)
            for f in range(CM):
                wz = mtile_dma(32 + f)
                wc_ = mtile_dma(40 + f)
                for q in range(2):
                    col0 = 1024 + 512 * q
                    pz = ps.tile([128, 512], F32, tag="ps")
                    for k in range(KC):
                        nc.tensor.matmul(pz, wz[:, k, :], xt_leaf[q][:, k, :],
                                         start=(k == 0), stop=(k == KC - 1))
                    pc = ps.tile([128, 512], F32, tag="ps")
                    for k in range(KC):
                        nc.tensor.matmul(pc, wc_[:, k, :], xt_leaf[q][:, k, :],
                                         start=(k == 0), stop=(k == KC - 1))
                    zs = wrk.tile([128, 512], F32, tag="zs", bufs=1)
                    nc.scalar.activation(zs, pz, SIG, bias=bias_sb[:, 32 + f:33 + f])
                    cs = wrk.tile([128, 512], F32, tag="cs", bufs=1)
                    nc.scalar.activation(cs, pc, TANH, bias=bias_sb[:, 40 + f:41 + f])
                    h_t = wrk.tile([128, 512], F32, tag="h")
                    nc.vector.tensor_mul(h_t, zs, cs)
                    nc.sync.dma_start(out=hT[f * 128:(f + 1) * 128, col0:col0 + 512], in_=h_t)
                    if q == 0:
                        # nodes 64..95 -> hbuf[6] slots 1..32
                        nc.vector.tensor_copy(
                            hbuf[6][:, f, 1:33, :],
                            h_t.rearrange("p (n b) -> p n b", b=BL),
                        )
                    else:
                        # nodes 96..126 -> slots 33..63 ; node 127 -> h127
                        nc.vector.tensor_copy(
                            hbuf[6][:, f, 33:64, :],
                            h_t[:, :496].rearrange("p (n b) -> p n b", b=BL),
                        )
                        nc.vector.tensor_copy(h127[:, f, :], h_t[:, 496:512])

            # residents: emitted after the leaf phase so leaf-stream DMAs go first
            for k in range(KC):
                nc.sync.dma_start(out=wgl_sb[:, k, :], in_=wgl[k * 128:(k + 1) * 128, :])
                nc.sync.dma_start(out=wcl_sb[:, k, :], in_=wcl[k * 128:(k + 1) * 128, :])
                nc.sync.dma_start(out=wcr_sb[:, k, :], in_=wcr[k * 128:(k + 1) * 128, :])

            # ---------------- phase A: internal x-projections (nodes 0..63) ----------------
            xt_int = []
            for hh in range(2):
                xt_t = xtp.tile([128, KC, 512], BF, tag="xt", name=f"xti{hh}")
                nc.sync.dma_start(out=xt_t, in_=xT[hh])
                xt_int.append(xt_t)
            for m in range(TM):
                wt = mtile_dma(m)
                for hh in (1, 0):
                    col0 = 512 * hh
                    pm = ps.tile([128, 512], F32, tag="ps")
                    for k in range(KC):
                        nc.tensor.matmul(pm, wt[:, k, :], xt_int[hh][:, k, :],
                                         start=(k == 0), stop=(k == KC - 1))
                    xg_sb = xio.tile([128, 512], BF, tag="xio")
                    nc.vector.tensor_copy(xg_sb, pm)
                    nc.sync.dma_start(out=xgd[m * 128:(m + 1) * 128, col0:col0 + 512], in_=xg_sb)
                    if hh == 1:
                        # node 63 cols are 1008..1023 -> offset 496 in this half
                        nc.vector.tensor_copy(x63_sb[:, m, :], pm[:, 496:512])

            # ---------------- node 63 (lh = h127, rh = 0) ----------------
            rl = gat.tile([128, KC, 256], BF, tag="rl")
            zl = gat.tile([128, KC, 256], BF, tag="zl")
            zz = gat.tile([128, KC, 256], BF, tag="zz")
            gmap63 = {0: rl, 2: zl, 4: zz}
            for m in range(GM):
                g = m // 8
                if g in (1, 3):  # rr, zr multiply rh = 0
                    continue
                pm = ps.tile([128, BL], F32, tag="ps")
                for k in range(KC):
                    nc.tensor.matmul(pm, wgl_sb[:, k, m * 128:(m + 1) * 128], h127[:, k, :],
                                     start=(k == 0), stop=(k == KC - 1))
                nc.vector.tensor_add(pm, pm, x63_sb[:, m, :])
                nc.scalar.activation(gmap63[g][:, m % 8, :BL], pm, SIG,
                                     bias=bias_sb[:, m:m + 1])
            pl = rl
            for k in range(KC):
                nc.vector.tensor_mul(pl[:, k, :BL], rl[:, k, :BL], h127[:, k, :])
            for f in range(CM):
                pm = ps.tile([128, BL], F32, tag="ps")
                for k in range(KC):
                    nc.tensor.matmul(pm, wcl_sb[:, k, f * 128:(f + 1) * 128], pl[:, k, :BL],
                                     start=(k == 0))
                nc.tensor.matmul(pm, ident_sb, x63_sb[:, 40 + f, :], start=False, stop=True)
                cell = wrk.tile([128, 256], F32, tag="cell", bufs=1)
                nc.scalar.activation(cell[:, :BL], pm, TANH, bias=bias_sb[:, 40 + f:41 + f])
                h_t = wrk.tile([128, 512], F32, tag="h")
                tmp = wrk.tile([128, 512], F32, tag="tmp")
                nc.vector.tensor_mul(h_t[:, :BL], zl[:, f, :BL], h127[:, f, :])
                nc.vector.tensor_mul(tmp[:, :BL], zz[:, f, :BL], cell[:, :BL])
                nc.vector.tensor_add(h_t[:, :BL], h_t[:, :BL], tmp[:, :BL])
                nc.sync.dma_start(out=hT[f * 128:(f + 1) * 128, 63 * BL:64 * BL], in_=h_t[:, :BL])
                nc.vector.tensor_copy(hbuf[6][:, f, 0, :], h_t[:, :BL])

            # ---------------- levels 5..0 ----------------
            for d in range(5, -1, -1):
                node0 = 2 ** d - 1
                n_nodes = 2 ** d
                child = hbuf[d + 1].rearrange("p k (n two) b -> p k n two b", two=2)
                n_splits = max(1, (n_nodes * BL) // 256) if n_nodes * BL >= 256 else 1
                nodes_per_split = (n_nodes + n_splits - 1) // n_splits
                for s in range(n_splits):
                    ns0 = s * nodes_per_split
                    ns = min(nodes_per_split, n_nodes - ns0)
                    C = ns * BL
                    c0 = (node0 + ns0) * BL  # global hT/xgd column base
                    rl = gat.tile([128, KC, 256], BF, tag="rl")
                    rr = gat.tile([128, KC, 256], BF, tag="rr")
                    zl = gat.tile([128, KC, 256], BF, tag="zl")
                    zr = gat.tile([128, KC, 256], BF, tag="zr")
                    zz = gat.tile([128, KC, 256], BF, tag="zz")
                    gmap = {0: rl, 1: rr, 2: zl, 3: zr, 4: zz}

                    def LH(k):
                        return child[:, k, ns0:ns0 + ns, 0, :]

                    def RH(k):
                        return child[:, k, ns0:ns0 + ns, 1, :]

                    for m in range(GM):
                        wgrt = wst.tile([128, KC, 128], BF, tag="w")
                        nc.sync.dma_start(out=wgrt, in_=wgr[m])
                        pm = ps.tile([128, C], F32, tag="ps")
                        for k in range(KC):
                            nc.tensor.matmul(pm, wgl_sb[:, k, m * 128:(m + 1) * 128], LH(k),
                                             start=(k == 0), stop=False)
                        for k in range(KC):
                            nc.tensor.matmul(pm, wgrt[:, k, :], RH(k), start=False,
                                             stop=(k == KC - 1))
                        xgt = xio.tile([128, 512], BF, tag="xio")
                        nc.sync.dma_start(out=xgt[:, :C], in_=xgd[m * 128:(m + 1) * 128, c0:c0 + C])
                        nc.vector.tensor_add(pm, pm, xgt[:, :C])
                        nc.scalar.activation(gmap[m // 8][:, m % 8, :C], pm, SIG,
                                             bias=bias_sb[:, m:m + 1])
                    pl, pr = rl, rr
                    for k in range(KC):
                        nc.vector.tensor_mul(
                            pl[:, k, :C].rearrange("p (n b) -> p n b", b=BL),
                            rl[:, k, :C].rearrange("p (n b) -> p n b", b=BL), LH(k))
                        nc.vector.tensor_mul(
                            pr[:, k, :C].rearrange("p (n b) -> p n b", b=BL),
                            rr[:, k, :C].rearrange("p (n b) -> p n b", b=BL), RH(k))
                    for f in range(CM):
                        pm = ps.tile([128, C], F32, tag="ps")
                        for k in range(KC):
                            nc.tensor.matmul(pm, wcl_sb[:, k, f * 128:(f + 1) * 128], pl[:, k, :C],
                                             start=(k == 0), stop=False)
                        for k in range(KC):
                            nc.tensor.matmul(pm, wcr_sb[:, k, f * 128:(f + 1) * 128], pr[:, k, :C],
                                             start=False, stop=(k == KC - 1))
                        xct = xio.tile([128, 512], BF, tag="xio")
                        nc.sync.dma_start(out=xct[:, :C],
                                          in_=xgd[(40 + f) * 128:(41 + f) * 128, c0:c0 + C])
                        nc.vector.tensor_add(pm, pm, xct[:, :C])
                        cell = wrk.tile([128, 256], F32, tag="cell", bufs=1)
                        nc.scalar.activation(cell[:, :C], pm, TANH, bias=bias_sb[:, 40 + f:41 + f])
                        h_t = wrk.tile([128, 512], F32, tag="h")
                        tmp = wrk.tile([128, 512], F32, tag="tmp")
                        nc.vector.tensor_mul(
                            h_t[:, :C].rearrange("p (n b) -> p n b", b=BL),
                            zl[:, f, :C].rearrange("p (n b) -> p n b", b=BL), LH(f))
                        nc.vector.tensor_mul(
                            tmp[:, :C].rearrange("p (n b) -> p n b", b=BL),
                            zr[:, f, :C].rearrange("p (n b) -> p n b", b=BL), RH(f))
                        nc.vector.tensor_add(h_t[:, :C], h_t[:, :C], tmp[:, :C])
                        nc.vector.tensor_mul(tmp[:, :C], zz[:, f, :C], cell[:, :C])
                        nc.vector.tensor_add(h_t[:, :C], h_t[:, :C], tmp[:, :C])
                        nc.sync.dma_start(out=hT[f * 128:(f + 1) * 128, c0:c0 + C],
                                          in_=h_t[:, :C])
                        nc.vector.tensor_copy(
                            hbuf[d][:, f, ns0:ns0 + ns, :],
                            h_t[:, :C].rearrange("p (n b) -> p n b", b=BL))
    return nc


def _build():
    global _BUILT
    if _BUILT is None:
        _BUILT = _build_nc()
    return _BUILT


def _make_in_maps(inp):
    bf = ml_dtypes.bfloat16
    x = inp["inputs"].astype(np.float32)  # [L, B, I]

    W6 = np.concatenate([inp["Wg_ih"], inp["Wc_ih"]], axis=0)  # [6H, I]
    # tile layout [m, p, k, c]: element = W.T[k*128+p, m*128+c]
    wih_np = np.ascontiguousarray(
        W6.reshape(TM, 128, KC, 128).transpose(0, 3, 2, 1)).astype(bf)
    wgl_np = np.ascontiguousarray(inp["Wg_l"].T).astype(bf)   # [H, 5H]
    wgr_np = np.ascontiguousarray(
        inp["Wg_r"].reshape(GM, 128, KC, 128).transpose(0, 3, 2, 1)).astype(bf)
    wcl_np = np.ascontiguousarray(inp["Wc_l"].T).astype(bf)   # [H, H]
    wcr_np = np.ascontiguousarray(inp["Wc_r"].T).astype(bf)
    bias_np = np.ascontiguousarray(
        np.concatenate([inp["bg_ih"], inp["bc_ih"]]).astype(np.float32)
        .reshape(TM, 128).T)  # [128, 48]

    in_maps = []
    for c in range(NCORES):
        xs = x[:, c * BL:(c + 1) * BL, :]  # [L, BL, I]
        xTc = xs.transpose(2, 0, 1).reshape(I, COLS)  # [I, cols]
        xTc = np.ascontiguousarray(
            xTc.reshape(KC, 128, 4, 512).transpose(2, 1, 0, 3)).astype(bf)
        in_maps.append(dict(xT=xTc, wih=wih_np, wgl=wgl_np, wgr=wgr_np,
                            wcl=wcl_np, wcr=wcr_np, bias=bias_np))
    return in_maps


def kernel(**inputs):
    inp = {k: np.asarray(v) for k, v in inputs.items()}
    in_maps = _make_in_maps(inp)
    nc = _build()
    from concourse.bass_utils import run_bass_kernel_spmd
    res = run_bass_kernel_spmd(nc, in_maps, list(range(NCORES)))

    parts = []
    for c in range(NCORES):
        hTc = np.asarray(res.results[c]["hT"], dtype=np.float32)  # [H, COLS]
        parts.append(hTc.reshape(H, L, BL).transpose(2, 1, 0))    # [BL, L, H]
    Hfin = np.ascontiguousarray(np.concatenate(parts, axis=0))    # [B, L, H]
    return Hfin, np.zeros((B, H), np.float32)


# revision 16
# speedup vs baseline: 1.3063x; 1.0648x over previous
"""Tree-GRU (binary heap, L=128) Trainium2 kernel.

Strategy:
  - Data-parallel over batch: B=128 -> 16 examples per NeuronCore, 8 cores.
  - The tree is a fixed binary heap (node n children 2n+1/2n+2), so nodes are
    processed level-by-level: leaves (64..127) need no recurrent matmuls at
    all; then node 63 (single real child 127), then levels 5..0.
  - Everything on-chip is feature-major [feat, node*16+b]; weights are
    pre-transposed on the host so no on-chip transposes are needed.
  - bf16 matmuls with fp32 PSUM accumulation; x-projections for internal
    nodes staged through DRAM in bf16; leaf hiddens fused straight from PSUM.
"""
import sys

sys.path.insert(0, "/opt/trn_rl_repo")

import numpy as np
import ml_dtypes

L = 128
B = 128
I = 1024
H = 1024
NCORES = 8
BL = B // NCORES          # 16 examples per core
COLS = L * BL             # 2048 (node*16 + b)
KC = I // 128             # 8 contraction chunks
GM = 5 * H // 128         # 40 gate M-tiles
CM = H // 128             # 8 cell M-tiles
TM = GM + CM              # 48
INT_COLS = 64 * BL        # 1024 internal-node columns (nodes 0..63)

_BUILT = None


def _build_nc():
    import concourse.bass as bass
    import concourse.mybir as mybir
    import concourse.tile as tile
    from concourse.vector_clock import ScopedClock, VectorClock

    BF = mybir.dt.bfloat16
    F32 = mybir.dt.float32
    SIG = mybir.ActivationFunctionType.Sigmoid
    TANH = mybir.ActivationFunctionType.Tanh

    def _legalize_single_wait(nc):
        # The walrus in this container encodes at most ONE sync wait per
        # TPB instruction; Tile's wait assigner can attach several. Hoist
        # the extras onto fresh same-engine NoOps placed directly before
        # the over-subscribed instruction.
        cur = nc.cur_bb.bb.instructions
        for blk in nc.m.functions[0].blocks:
            lst = blk.instructions
            i = 0
            while i < len(lst):
                ins = lst[i]
                si = getattr(ins, "sync_info", None)
                ow = list(si.on_wait) if si is not None and si.on_wait else []
                if len(ow) > 1:
                    si.on_wait = [ow[-1]]
                    for w in ow[:-1]:
                        nop = nc.engines[ins.engine].nop()
                        nopins = nop.ins
                        popped = cur.pop()
                        assert popped.name == nopins.name
                        nopins.sync_info = mybir.SyncInfo(on_wait=[w], on_update=[])
                        lst.insert(i, nopins)
                        i += 1
                i += 1

    class TC(tile.TileContext):
        def _drain_and_barrier(self, tick_clock, wait_clock):
            super()._drain_and_barrier(tick_clock, wait_clock)
            _legalize_single_wait(self.nc)

    nc = bass.Bass(trn_type="TRN2")

    xT = nc.declare_dram_parameter("xT", [4, 128, KC, 512], BF, isOutput=False)
    wih = nc.declare_dram_parameter("wih", [TM, 128, KC, 128], BF, isOutput=False)
    wgl = nc.declare_dram_parameter("wgl", [H, 5 * H], BF, isOutput=False)
    wgr = nc.declare_dram_parameter("wgr", [GM, 128, KC, 128], BF, isOutput=False)
    wcl = nc.declare_dram_parameter("wcl", [H, H], BF, isOutput=False)
    wcr = nc.declare_dram_parameter("wcr", [H, H], BF, isOutput=False)
    bias = nc.declare_dram_parameter("bias", [128, TM], F32, isOutput=False)
    hT = nc.declare_dram_parameter("hT", [H, COLS], F32, isOutput=True)

    xgd = nc.dram_tensor("xgd", [6 * H, INT_COLS], BF)

    with TC(nc) as tc:
        with (
            tc.tile_pool(name="res", bufs=1) as res,
            tc.tile_pool(name="xtp", bufs=2) as xtp,
            tc.tile_pool(name="wst", bufs=5) as wst,
            tc.tile_pool(name="xio", bufs=5) as xio,
            tc.tile_pool(name="gat", bufs=1) as gat,
            tc.tile_pool(name="wrk", bufs=2) as wrk,
            tc.tile_pool(name="ps", bufs=8, space="PSUM") as ps,
        ):
            # ---------------- residents ----------------
            wgl_sb = res.tile([128, KC, 5 * H], BF, tag="wgl")
            wcl_sb = res.tile([128, KC, H], BF, tag="wcl")
            wcr_sb = res.tile([128, KC, H], BF, tag="wcr")
            bias_sb = res.tile([128, TM], F32, tag="bias")
            nc.sync.dma_start(out=bias_sb, in_=bias[:, :])
            x63_sb = res.tile([128, TM, BL], BF, tag="x63")
            h127 = res.tile([128, KC, BL], BF, tag="h127")
            # per-level hidden buffers (bf16), slot j of hbuf[d] = node (2^d-1)+j
            hbuf = {}
            for d in range(7):
                hbuf[d] = res.tile([128, KC, max(2 ** d, 1), BL], BF,
                                   tag=f"h{d}", name=f"hbuf{d}")

            def mtile_dma(m):
                wt = wst.tile([128, KC, 128], BF, tag="w")
                nc.sync.dma_start(out=wt, in_=wih[m])
                return wt

            # ---------------- phase A: leaves (nodes 64..127) ----------------
            # h_leaf = sigmoid(z) * tanh(c); no recurrent terms. Single pass
            # over the z/c weight tiles, both 512-col quarters per tile.
            xt_leaf = []
            for q in range(2):
                xt_t = xtp.tile([128, KC, 512], BF, tag="xt", name=f"xtl{q}")
                nc.sync.dma_start(out=xt_t, in_=xT[2 + q])
                xt_leaf.append(xt_t)
            for f in range(CM):
                wz = mtile_dma(32 + f)
                wc_ = mtile_dma(40 + f)
                for q in range(2):
                    col0 = 1024 + 512 * q
                    pz = ps.tile([128, 512], F32, tag="ps")
                    for k in range(KC):
                        nc.tensor.matmul(pz, wz[:, k, :], xt_leaf[q][:, k, :],
                                         start=(k == 0), stop=(k == KC - 1))
                    pc = ps.tile([128, 512], F32, tag="ps")
                    for k in range(KC):
                        nc.tensor.matmul(pc, wc_[:, k, :], xt_leaf[q][:, k, :],
                                         start=(k == 0), stop=(k == KC - 1))
                    zs = wrk.tile([128, 512], F32, tag="zs", bufs=1)
                    nc.scalar.activation(zs, pz, SIG, bias=bias_sb[:, 32 + f:33 + f])
                    cs = wrk.tile([128, 512], F32, tag="cs", bufs=1)
                    nc.scalar.activation(cs, pc, TANH, bias=bias_sb[:, 40 + f:41 + f])
                    h_t = wrk.tile([128, 512], F32, tag="h")
                    nc.vector.tensor_mul(h_t, zs, cs)
                    nc.sync.dma_start(out=hT[f * 128:(f + 1) * 128, col0:col0 + 512], in_=h_t)
                    if q == 0:
                        # nodes 64..95 -> hbuf[6] slots 1..32
                        nc.vector.tensor_copy(
                            hbuf[6][:, f, 1:33, :],
                            h_t.rearrange("p (n b) -> p n b", b=BL),
                        )
                    else:
                        # nodes 96..126 -> slots 33..63 ; node 127 -> h127
                        nc.vector.tensor_copy(
                            hbuf[6][:, f, 33:64, :],
                            h_t[:, :496].rearrange("p (n b) -> p n b", b=BL),
                        )
                        nc.vector.tensor_copy(h127[:, f, :], h_t[:, 496:512])

            # residents: emitted after the leaf phase so leaf-stream DMAs go first
            for k in range(KC):
                nc.sync.dma_start(out=wgl_sb[:, k, :], in_=wgl[k * 128:(k + 1) * 128, :])
                nc.sync.dma_start(out=wcl_sb[:, k, :], in_=wcl[k * 128:(k + 1) * 128, :])
                nc.sync.dma_start(out=wcr_sb[:, k, :], in_=wcr[k * 128:(k + 1) * 128, :])

            # ---------------- phase A: internal x-projections (nodes 0..63) ----------------
            xt_int = []
            for hh in range(2):
                xt_t = xtp.tile([128, KC, 512], BF, tag="xt", name=f"xti{hh}")
                nc.sync.dma_start(out=xt_t, in_=xT[hh])
                xt_int.append(xt_t)
            for m in range(TM):
                wt = mtile_dma(m)
                for hh in (1, 0):
                    col0 = 512 * hh
                    pm = ps.tile([128, 512], F32, tag="ps")
                    for k in range(KC):
                        nc.tensor.matmul(pm, wt[:, k, :], xt_int[hh][:, k, :],
                                         start=(k == 0), stop=(k == KC - 1))
                    xg_sb = xio.tile([128, 512], BF, tag="xio")
                    nc.vector.tensor_copy(xg_sb, pm)
                    nc.sync.dma_start(out=xgd[m * 128:(m + 1) * 128, col0:col0 + 512], in_=xg_sb)
                    if hh == 1:
                        # node 63 cols are 1008..1023 -> offset 496 in this half
                        nc.vector.tensor_copy(x63_sb[:, m, :], pm[:, 496:512])

            # ---------------- node 63 (lh = h127, rh = 0) ----------------
            rl = gat.tile([128, KC, 256], BF, tag="rl")
            zl = gat.tile([128, KC, 256], BF, tag="zl")
            zz = gat.tile([128, KC, 256], BF, tag="zz")
            gmap63 = {0: rl, 2: zl, 4: zz}
            for m in range(GM):
                g = m // 8
                if g in (1, 3):  # rr, zr multiply rh = 0
                    continue
                pm = ps.tile([128, BL], F32, tag="ps")
                for k in range(KC):
                    nc.tensor.matmul(pm, wgl_sb[:, k, m * 128:(m + 1) * 128], h127[:, k, :],
                                     start=(k == 0), stop=(k == KC - 1))
                nc.vector.tensor_add(pm, pm, x63_sb[:, m, :])
                nc.scalar.activation(gmap63[g][:, m % 8, :BL], pm, SIG,
                                     bias=bias_sb[:, m:m + 1])
            pl = rl
            for k in range(KC):
                nc.vector.tensor_mul(pl[:, k, :BL], rl[:, k, :BL], h127[:, k, :])
            for f in range(CM):
                pm = ps.tile([128, BL], F32, tag="ps")
                for k in range(KC):
                    nc.tensor.matmul(pm, wcl_sb[:, k, f * 128:(f + 1) * 128], pl[:, k, :BL],
                                     start=(k == 0))
                nc.tensor.matmul(pm, ident_sb, x63_sb[:, 40 + f, :], start=False, stop=True)
                cell = wrk.tile([128, 256], F32, tag="cell", bufs=1)
                nc.scalar.activation(cell[:, :BL], pm, TANH, bias=bias_sb[:, 40 + f:41 + f])
                h_t = wrk.tile([128, 512], F32, tag="h")
                tmp = wrk.tile([128, 256], F32, tag="tmp")
                nc.vector.tensor_mul(h_t[:, :BL], zl[:, f, :BL], h127[:, f, :])
                nc.vector.tensor_mul(tmp[:, :BL], zz[:, f, :BL], cell[:, :BL])
                nc.vector.tensor_add(h_t[:, :BL], h_t[:, :BL], tmp[:, :BL])
                nc.sync.dma_start(out=hT[f * 128:(f + 1) * 128, 63 * BL:64 * BL], in_=h_t[:, :BL])
                nc.vector.tensor_copy(hbuf[6][:, f, 0, :], h_t[:, :BL])

            # ---------------- levels 5..0 ----------------
            for d in range(5, -1, -1):
                node0 = 2 ** d - 1
                n_nodes = 2 ** d
                child = hbuf[d + 1].rearrange("p k (n two) b -> p k n two b", two=2)
                n_splits = max(1, (n_nodes * BL) // 256) if n_nodes * BL >= 256 else 1
                nodes_per_split = (n_nodes + n_splits - 1) // n_splits
                for s in range(n_splits):
                    ns0 = s * nodes_per_split
                    ns = min(nodes_per_split, n_nodes - ns0)
                    C = ns * BL
                    c0 = (node0 + ns0) * BL  # global hT/xgd column base
                    rl = gat.tile([128, KC, 256], BF, tag="rl")
                    rr = gat.tile([128, KC, 256], BF, tag="rr")
                    zl = gat.tile([128, KC, 256], BF, tag="zl")
                    zr = gat.tile([128, KC, 256], BF, tag="zr")
                    zz = gat.tile([128, KC, 256], BF, tag="zz")
                    gmap = {0: rl, 1: rr, 2: zl, 3: zr, 4: zz}

                    def LH(k):
                        return child[:, k, ns0:ns0 + ns, 0, :]

                    def RH(k):
                        return child[:, k, ns0:ns0 + ns, 1, :]

                    for m in range(GM):
                        wgrt = wst.tile([128, KC, 128], BF, tag="w")
                        nc.gpsimd.dma_start(out=wgrt, in_=wgr[m])
                        pm = ps.tile([128, C], F32, tag="ps")
                        for k in range(KC):
                            nc.tensor.matmul(pm, wgl_sb[:, k, m * 128:(m + 1) * 128], LH(k),
                                             start=(k == 0), stop=False)
                        for k in range(KC):
                            nc.tensor.matmul(pm, wgrt[:, k, :], RH(k), start=False,
                                             stop=(k == KC - 1))
                        xgt = xio.tile([128, 512], BF, tag="xio")
                        nc.sync.dma_start(out=xgt[:, :C], in_=xgd[m * 128:(m + 1) * 128, c0:c0 + C])
                        nc.vector.tensor_add(pm, pm, xgt[:, :C])
                        nc.scalar.activation(gmap[m // 8][:, m % 8, :C], pm, SIG,
                                             bias=bias_sb[:, m:m + 1])
                    pl, pr = rl, rr
                    for k in range(KC):
                        nc.vector.tensor_mul(
                            pl[:, k, :C].rearrange("p (n b) -> p n b", b=BL),
                            rl[:, k, :C].rearrange("p (n b) -> p n b", b=BL), LH(k))
                        nc.vector.tensor_mul(
                            pr[:, k, :C].rearrange("p (n b) -> p n b", b=BL),
                            rr[:, k, :C].rearrange("p (n b) -> p n b", b=BL), RH(k))
                    for f in range(CM):
                        pm = ps.tile([128, C], F32, tag="ps")
                        for k in range(KC):
                            nc.tensor.matmul(pm, wcl_sb[:, k, f * 128:(f + 1) * 128], pl[:, k, :C],
                                             start=(k == 0), stop=False)
                        for k in range(KC):
                            nc.tensor.matmul(pm, wcr_sb[:, k, f * 128:(f + 1) * 128], pr[:, k, :C],
                                             start=False, stop=(k == KC - 1))
                        xct = xio.tile([128, 512], BF, tag="xio")
                        nc.sync.dma_start(out=xct[:, :C],
                                          in_=xgd[(40 + f) * 128:(41 + f) * 128, c0:c0 + C])
                        nc.vector.tensor_add(pm, pm, xct[:, :C])
                        cell = wrk.tile([128, 256], F32, tag="cell", bufs=1)
                        nc.scalar.activation(cell[:, :C], pm, TANH, bias=bias_sb[:, 40 + f:41 + f])
                        h_t = wrk.tile([128, 512], F32, tag="h")
                        tmp = wrk.tile([128, 256], F32, tag="tmp")
                        nc.vector.tensor_mul(
                            h_t[:, :C].rearrange("p (n b) -> p n b", b=BL),
                            zl[:, f, :C].rearrange("p (n b) -> p n b", b=BL), LH(f))
                        nc.vector.tensor_mul(
                            tmp[:, :C].rearrange("p (n b) -> p n b", b=BL),
                            zr[:, f, :C].rearrange("p (n b) -> p n b", b=BL), RH(f))
                        nc.vector.tensor_add(h_t[:, :C], h_t[:, :C], tmp[:, :C])
                        nc.vector.tensor_mul(tmp[:, :C], zz[:, f, :C], cell[:, :C])
                        nc.vector.tensor_add(h_t[:, :C], h_t[:, :C], tmp[:, :C])
                        nc.sync.dma_start(out=hT[f * 128:(f + 1) * 128, c0:c0 + C],
                                          in_=h_t[:, :C])
                        nc.vector.tensor_copy(
                            hbuf[d][:, f, ns0:ns0 + ns, :],
                            h_t[:, :C].rearrange("p (n b) -> p n b", b=BL))
    return nc


def _build():
    global _BUILT
    if _BUILT is None:
        _BUILT = _build_nc()
    return _BUILT


def _make_in_maps(inp):
    bf = ml_dtypes.bfloat16
    x = inp["inputs"].astype(np.float32)  # [L, B, I]

    W6 = np.concatenate([inp["Wg_ih"], inp["Wc_ih"]], axis=0)  # [6H, I]
    # tile layout [m, p, k, c]: element = W.T[k*128+p, m*128+c]
    wih_np = np.ascontiguousarray(
        W6.reshape(TM, 128, KC, 128).transpose(0, 3, 2, 1)).astype(bf)
    wgl_np = np.ascontiguousarray(inp["Wg_l"].T).astype(bf)   # [H, 5H]
    wgr_np = np.ascontiguousarray(
        inp["Wg_r"].reshape(GM, 128, KC, 128).transpose(0, 3, 2, 1)).astype(bf)
    wcl_np = np.ascontiguousarray(inp["Wc_l"].T).astype(bf)   # [H, H]
    wcr_np = np.ascontiguousarray(inp["Wc_r"].T).astype(bf)
    bias_np = np.ascontiguousarray(
        np.concatenate([inp["bg_ih"], inp["bc_ih"]]).astype(np.float32)
        .reshape(TM, 128).T)  # [128, 48]

    in_maps = []
    for c in range(NCORES):
        xs = x[:, c * BL:(c + 1) * BL, :]  # [L, BL, I]
        xTc = xs.transpose(2, 0, 1).reshape(I, COLS)  # [I, cols]
        xTc = np.ascontiguousarray(
            xTc.reshape(KC, 128, 4, 512).transpose(2, 1, 0, 3)).astype(bf)
        in_maps.append(dict(xT=xTc, wih=wih_np, wgl=wgl_np, wgr=wgr_np,
                            wcl=wcl_np, wcr=wcr_np, bias=bias_np))
    return in_maps


def kernel(**inputs):
    inp = {k: np.asarray(v) for k, v in inputs.items()}
    in_maps = _make_in_maps(inp)
    nc = _build()
    from concourse.bass_utils import run_bass_kernel_spmd
    res = run_bass_kernel_spmd(nc, in_maps, list(range(NCORES)))

    parts = []
    for c in range(NCORES):
        hTc = np.asarray(res.results[c]["hT"], dtype=np.float32)  # [H, COLS]
        parts.append(hTc.reshape(H, L, BL).transpose(2, 1, 0))    # [BL, L, H]
    Hfin = np.ascontiguousarray(np.concatenate(parts, axis=0))    # [B, L, H]
    return Hfin, np.zeros((B, H), np.float32)
